# revision 8
# baseline (speedup 1.0000x reference)
"""Trainium2 Bass kernel for sorted-segment sum+mean (segment_reduce).

out[g] = concat(mean_g, sum_g) over rows of nbr_fea grouped by sorted
segment_ids; out shape [num_segments, 2*D].

Strategy
--------
Rows are sorted by segment id, so each segment is a contiguous row range.
Segments are grouped into "chunks" of S=32 consecutive segments; each chunk's
rows are packed (on host) into T row-tiles of 128 rows, laid out DMA-optimally
as [chunk][partition][tile][feat] so chunk loads are fully contiguous DMAs
(two chunks batched per dma_start).

The f32 features are split on host into an exact bf16 hi/lo pair
(x = hi + lo + O(2^-18 x)), shipped side by side — same byte volume as f32 —
so the TensorEngine runs single-pass bf16 matmuls instead of 4x-slower fp32.

On device, per 128-row tile, a one-hot matrix U[row, slot] = (rel_id == slot)
is built on the VectorEngine (is_equal against an iota constant) in bf16 and
used as the matmul *stationary* operand (LDWEIGHTS of 32 cols, FWL-fast);
the moving operand is the [128 rows, hi|lo = 128] tile:
    psum[slot, 0:64]  += U.T @ hi
    psum[slot, 64:128]+= U.T @ lo
accumulated over the chunk's tiles in PSUM (fp32).  The epilogue adds the two
halves (exact sum; ACT stages the lo half since walrus allows one PSUM operand
per op), scales by host-baked 1/count on GpSimd for the mean, and stages
results in SBUF, DMA'd out in quarters.  Padding rows carry rel_id = -1 so
their one-hot row is all zero.

The kernel is compiled AFTER seeing the inputs, so the (data-dependent) tile
count per chunk is a compile-time constant; one SPMD program runs on all 8
cores (each core owns C=128 chunks = 4096 segments).
"""

import ml_dtypes
import numpy as np

import concourse.bass as bass
import concourse.mybir as mybir
import concourse.tile as tile
from concourse import bass_utils

N_TOTAL = 4_194_304
D = 64                       # feature dim
G = 32_768                   # num segments
N_CORES = 8
S = 32                       # segment slots per chunk (psum partitions)
C_TOTAL = G // S             # 1024 chunks
C = C_TOTAL // N_CORES       # 128 chunks per core
P = 128                      # rows per tile == SBUF partitions
DMA_BATCH = 2                # chunks per x dma_start

F32 = mybir.dt.float32
BF16 = mybir.dt.bfloat16
NP_BF16 = ml_dtypes.bfloat16


def _split_syncs(nc, max_waits=1):
    """This container's walrus accepts at most one sync-wait per instruction;
    split extra waits onto preceding same-engine NoOps (engine stalls at each
    wait in turn, so semantics are identical)."""
    n_split = 0
    for f in nc.m.functions:
        for bb in f.blocks:
            new_insts = []
            for ins in bb.instructions:
                si = getattr(ins, "sync_info", None)
                waits = list(si.on_wait) if si is not None and si.on_wait else []
                if len(waits) > max_waits:
                    n_split += 1
                    extra = waits[:-max_waits]
                    for i in range(0, len(extra), max_waits):
                        nop = mybir.InstNoOp(
                            name=f"{ins.name}_wsplit{i}", ins=[], outs=[]
                        )
                        nop.engine = ins.engine
                        nop.sync_info = mybir.SyncInfo(
                            on_wait=extra[i : i + max_waits], on_update=[]
                        )
                        new_insts.append(nop)
                    si.on_wait = waits[-max_waits:]
                new_insts.append(ins)
            bb.instructions = new_insts
    return n_split


def _build_bass(T, split_syncs=True):
    """Build the SPMD program for T row-tiles per chunk."""
    nc = bass.Bass("TRN2", debug=False, num_devices=1)

    x_d = nc.dram_tensor("x", [C, P, T * 2 * D], BF16, kind="ExternalInput")
    rel_d = nc.dram_tensor("rel", [P, C * T], BF16, kind="ExternalInput")
    iota_d = nc.dram_tensor("iota", [P, T * S], BF16, kind="ExternalInput")
    recip_d = nc.dram_tensor("recip", [S, C], F32, kind="ExternalInput")
    out_d = nc.dram_tensor("out", [S, C * 2 * D], F32, kind="ExternalOutput")

    QUARTER = C // 4

    with tile.TileContext(nc) as tc:
        with (
            tc.tile_pool(name="const", bufs=1) as const_pool,
            tc.tile_pool(name="xin", bufs=3) as x_pool,
            tc.tile_pool(name="oh", bufs=3) as oh_pool,
            tc.tile_pool(name="outs", bufs=1) as out_pool,
            tc.tile_pool(name="scr", bufs=3) as scr_pool,
            tc.tile_pool(name="ps", bufs=4, space="PSUM") as ps_pool,
        ):
            # constants go through the SWDGE (gpsimd) queue so the HWDGE
            # rings start streaming x immediately
            rel_sb = const_pool.tile([P, C * T], BF16)
            nc.gpsimd.dma_start(rel_sb[:], rel_d[:])
            iota_sb = const_pool.tile([P, T * S], BF16)
            nc.gpsimd.dma_start(iota_sb[:], iota_d[:])
            recip_sb = const_pool.tile([S, C], F32)
            nc.gpsimd.dma_start(recip_sb[:], recip_d[:])
            out_sb = out_pool.tile([S, C * 2 * D], F32)

            xt = None
            for c in range(C):
                b = c % DMA_BATCH
                if b == 0:
                    nb = min(DMA_BATCH, C - c)
                    xt = x_pool.tile([P, nb, T * 2 * D], BF16)
                    dma_eng = nc.sync if (c // DMA_BATCH) % 2 == 0 else nc.scalar
                    dma_eng.dma_start(
                        xt[:],
                        x_d[c : c + nb, :, :].rearrange("c p f -> p c f"),
                    )
                oh = oh_pool.tile([P, T * S], BF16)
                nc.vector.tensor_tensor(
                    oh[:],
                    rel_sb[:, c * T : (c + 1) * T].to_broadcast((P, T, S)),
                    iota_sb[:],
                    mybir.AluOpType.is_equal,
                )
                ps = ps_pool.tile([S, 2 * D], F32)
                for t in range(T):
                    nc.tensor.matmul(
                        ps[:],
                        oh[:, t * S : (t + 1) * S],
                        xt[:, b, t * 2 * D : (t + 1) * 2 * D],
                        start=(t == 0),
                        stop=(t == T - 1),
                    )
                base = c * 2 * D
                # exact sum = hi-part + lo-part (walrus allows only one PSUM
                # operand per op: stage the lo half through SBUF via ACT)
                lo_sb = scr_pool.tile([S, D], F32)
                nc.scalar.copy(lo_sb[:], ps[:, D : 2 * D])
                nc.vector.tensor_tensor(
                    out_sb[:, base + D : base + 2 * D],
                    ps[:, 0:D],
                    lo_sb[:],
                    mybir.AluOpType.add,
                )
                # mean = sum * (1/count)   (per-partition scalar, on GpSimd)
                nc.gpsimd.tensor_scalar(
                    out_sb[:, base : base + D],
                    out_sb[:, base + D : base + 2 * D],
                    recip_sb[:, c : c + 1],
                    None,
                    mybir.AluOpType.mult,
                )
                if (c + 1) % QUARTER == 0:
                    q0 = (c + 1 - QUARTER) * 2 * D
                    q1 = (c + 1) * 2 * D
                    nc.sync.dma_start(out_d[:, q0:q1], out_sb[:, q0:q1])

    if split_syncs:
        _split_syncs(nc)
    return nc


def _plan_and_pack(x, seg):
    """Host-side: chunk boundaries, tile count, packed/padded device arrays."""
    x = np.ascontiguousarray(x, dtype=np.float32)
    seg = np.asarray(seg).astype(np.int64)

    starts = np.searchsorted(seg, np.arange(0, G + 1, S)).astype(np.int64)
    n_rows = np.diff(starts)
    T = max(1, int(-(-int(n_rows.max()) // P)))  # ceil

    counts = np.bincount(seg, minlength=G).astype(np.float64)
    recip = (1.0 / np.maximum(counts, 1.0)).astype(np.float32)

    # row index for [chunk, partition, tile]: row = start_c + t*128 + p
    ridx = (
        starts[:-1][:, None, None]
        + np.arange(P, dtype=np.int64)[None, :, None]
        + (np.arange(T, dtype=np.int64) * P)[None, None, :]
    )
    valid = ridx < starts[1:][:, None, None]
    ridx_c = np.where(valid, ridx, 0)

    xg = x[ridx_c.reshape(-1)].reshape(C_TOTAL, P, T, D)
    xg[~valid] = 0.0
    hi = xg.astype(NP_BF16)
    lo = (xg - hi.astype(np.float32)).astype(NP_BF16)
    xbuf = np.empty((C_TOTAL, P, T, 2 * D), NP_BF16)
    xbuf[..., :D] = hi
    xbuf[..., D:] = lo
    del xg, hi, lo
    xbuf = xbuf.reshape(C_TOTAL, P, T * 2 * D)

    rel_all = (seg % S).astype(np.float32)
    relbuf = np.where(valid, rel_all[ridx_c], np.float32(-1.0)).astype(NP_BF16)

    iota_np = np.tile(np.arange(S, dtype=np.float32), (P, T)).astype(NP_BF16)

    in_maps = []
    for core in range(N_CORES):
        c0, c1 = core * C, (core + 1) * C
        rel_core = relbuf[c0:c1].transpose(1, 0, 2).reshape(P, C * T)
        recip_core = recip[core * C * S : (core + 1) * C * S].reshape(C, S).T
        in_maps.append(
            {
                "x": np.ascontiguousarray(xbuf[c0:c1]),
                "rel": np.ascontiguousarray(rel_core),
                "iota": iota_np,
                "recip": np.ascontiguousarray(recip_core),
            }
        )
    return T, in_maps


def _assemble(results):
    """[core]["out"] of shape [S, C*2*D] -> [G, 2*D]."""
    parts = []
    for core in range(N_CORES):
        v = results[core]["out"].reshape(S, C, 2, D)
        mean = v[:, :, 0, :].transpose(1, 0, 2).reshape(C * S, D)
        ssum = v[:, :, 1, :].transpose(1, 0, 2).reshape(C * S, D)
        parts.append(np.concatenate([mean, ssum], axis=1))
    return np.concatenate(parts, axis=0)


def _run_impl(nbr_fea, segment_ids, num_segments, trace=False, trace_kwargs=None):
    assert int(num_segments) == G, f"expected {G} segments, got {num_segments}"
    assert nbr_fea.shape == (N_TOTAL, D), nbr_fea.shape

    T, in_maps = _plan_and_pack(nbr_fea, segment_ids)
    nc = _build_bass(T)
    kw = {}
    if trace:
        kw = dict(trace=True, **(trace_kwargs or {}))
    res = bass_utils.run_bass_kernel_spmd(
        nc, in_maps, core_ids=list(range(N_CORES)), **kw
    )
    return _assemble(res.results), res


def kernel(nbr_fea, segment_ids, num_segments):
    out, _ = _run_impl(np.asarray(nbr_fea), np.asarray(segment_ids), num_segments)
    return out


# revision 9
# speedup vs baseline: 1.0600x; 1.0600x over previous
"""Trainium2 Bass kernel for sorted-segment sum+mean (segment_reduce).

out[g] = concat(mean_g, sum_g) over rows of nbr_fea grouped by sorted
segment_ids; out shape [num_segments, 2*D].

Strategy
--------
Rows are sorted by segment id, so each segment is a contiguous row range.
Segments are grouped into "chunks" of S=32 consecutive segments; each chunk's
rows are packed (on host) into T row-tiles of 128 rows, laid out DMA-optimally
as [chunk][partition][tile][feat] so chunk loads are fully contiguous DMAs
(two chunks batched per dma_start).

The f32 features are split on host into an exact bf16 hi/lo pair
(x = hi + lo + O(2^-18 x)), shipped side by side — same byte volume as f32 —
so the TensorEngine runs single-pass bf16 matmuls instead of 4x-slower fp32.

On device, per 128-row tile, a one-hot matrix U[row, slot] = (rel_id == slot)
is built on the VectorEngine (is_equal against an iota constant) in bf16 and
used as the matmul *stationary* operand (LDWEIGHTS of 32 cols, FWL-fast);
the moving operand is the [128 rows, hi|lo = 128] tile:
    psum[slot, 0:64]  += U.T @ hi
    psum[slot, 64:128]+= U.T @ lo
accumulated over the chunk's tiles in PSUM (fp32).  The epilogue adds the two
halves (exact sum; ACT stages the lo half since walrus allows one PSUM operand
per op), scales by host-baked 1/count on GpSimd for the mean, and stages
results in SBUF, DMA'd out in quarters.  Padding rows carry rel_id = -1 so
their one-hot row is all zero.

The kernel is compiled AFTER seeing the inputs, so the (data-dependent) tile
count per chunk is a compile-time constant; one SPMD program runs on all 8
cores (each core owns C=128 chunks = 4096 segments).
"""

import ml_dtypes
import numpy as np

import concourse.bass as bass
import concourse.mybir as mybir
import concourse.tile as tile
from concourse import bass_utils

N_TOTAL = 4_194_304
D = 64                       # feature dim
G = 32_768                   # num segments
N_CORES = 8
S = 32                       # segment slots per chunk (psum partitions)
C_TOTAL = G // S             # 1024 chunks
C = C_TOTAL // N_CORES       # 128 chunks per core
P = 128                      # rows per tile == SBUF partitions
DMA_BATCH = 2                # chunks per x dma_start

F32 = mybir.dt.float32
BF16 = mybir.dt.bfloat16
NP_BF16 = ml_dtypes.bfloat16


def _split_syncs(nc, max_waits=1):
    """This container's walrus accepts at most one sync-wait per instruction;
    split extra waits onto preceding same-engine NoOps (engine stalls at each
    wait in turn, so semantics are identical)."""
    n_split = 0
    for f in nc.m.functions:
        for bb in f.blocks:
            new_insts = []
            for ins in bb.instructions:
                si = getattr(ins, "sync_info", None)
                waits = list(si.on_wait) if si is not None and si.on_wait else []
                if len(waits) > max_waits:
                    n_split += 1
                    extra = waits[:-max_waits]
                    for i in range(0, len(extra), max_waits):
                        nop = mybir.InstNoOp(
                            name=f"{ins.name}_wsplit{i}", ins=[], outs=[]
                        )
                        nop.engine = ins.engine
                        nop.sync_info = mybir.SyncInfo(
                            on_wait=extra[i : i + max_waits], on_update=[]
                        )
                        new_insts.append(nop)
                    si.on_wait = waits[-max_waits:]
                new_insts.append(ins)
            bb.instructions = new_insts
    return n_split


def _build_bass(T, split_syncs=True):
    """Build the SPMD program for T row-tiles per chunk."""
    nc = bass.Bass("TRN2", debug=False, num_devices=1)

    x_d = nc.dram_tensor("x", [C, P, T * 2 * D], BF16, kind="ExternalInput")
    rel_d = nc.dram_tensor("rel", [P, C * T], BF16, kind="ExternalInput")
    iota_d = nc.dram_tensor("iota", [P, T * S], BF16, kind="ExternalInput")
    recip_d = nc.dram_tensor("recip", [S, C], F32, kind="ExternalInput")
    out_d = nc.dram_tensor("out", [S, C * 2 * D], F32, kind="ExternalOutput")

    QUARTER = C // 4

    with tile.TileContext(nc) as tc:
        with (
            tc.tile_pool(name="const", bufs=1) as const_pool,
            tc.tile_pool(name="xin", bufs=4) as x_pool,
            tc.tile_pool(name="oh", bufs=3) as oh_pool,
            tc.tile_pool(name="outs", bufs=1) as out_pool,
            tc.tile_pool(name="scr", bufs=3) as scr_pool,
            tc.tile_pool(name="ps", bufs=4, space="PSUM") as ps_pool,
        ):
            # constants go through the SWDGE (gpsimd) queue so the HWDGE
            # rings start streaming x immediately
            rel_sb = const_pool.tile([P, C * T], BF16)
            nc.gpsimd.dma_start(rel_sb[:], rel_d[:])
            iota_sb = const_pool.tile([P, T * S], BF16)
            nc.gpsimd.dma_start(iota_sb[:], iota_d[:])
            recip_sb = const_pool.tile([S, C], F32)
            nc.gpsimd.dma_start(recip_sb[:], recip_d[:])
            out_sb = out_pool.tile([S, C * 2 * D], F32)

            xt = None
            for c in range(C):
                b = c % DMA_BATCH
                if b == 0:
                    nb = min(DMA_BATCH, C - c)
                    xt = x_pool.tile([P, nb, T * 2 * D], BF16)
                    dma_eng = nc.sync if (c // DMA_BATCH) % 2 == 0 else nc.scalar
                    dma_eng.dma_start(
                        xt[:],
                        x_d[c : c + nb, :, :].rearrange("c p f -> p c f"),
                    )
                oh = oh_pool.tile([P, T * S], BF16)
                nc.vector.tensor_tensor(
                    oh[:],
                    rel_sb[:, c * T : (c + 1) * T].to_broadcast((P, T, S)),
                    iota_sb[:],
                    mybir.AluOpType.is_equal,
                )
                ps = ps_pool.tile([S, 2 * D], F32)
                for t in range(T):
                    nc.tensor.matmul(
                        ps[:],
                        oh[:, t * S : (t + 1) * S],
                        xt[:, b, t * 2 * D : (t + 1) * 2 * D],
                        start=(t == 0),
                        stop=(t == T - 1),
                    )
                base = c * 2 * D
                # exact sum = hi-part + lo-part (walrus allows only one PSUM
                # operand per op: stage the lo half through SBUF via ACT)
                lo_sb = scr_pool.tile([S, D], F32)
                nc.scalar.copy(lo_sb[:], ps[:, D : 2 * D])
                nc.vector.tensor_tensor(
                    out_sb[:, base + D : base + 2 * D],
                    ps[:, 0:D],
                    lo_sb[:],
                    mybir.AluOpType.add,
                )
                # mean = sum * (1/count)   (per-partition scale, on ACT)
                nc.scalar.activation(
                    out_sb[:, base : base + D],
                    out_sb[:, base + D : base + 2 * D],
                    mybir.ActivationFunctionType.Copy,
                    scale=recip_sb[:, c : c + 1],
                )
                if (c + 1) % QUARTER == 0:
                    q0 = (c + 1 - QUARTER) * 2 * D
                    q1 = (c + 1) * 2 * D
                    nc.sync.dma_start(out_d[:, q0:q1], out_sb[:, q0:q1])

    if split_syncs:
        _split_syncs(nc)
    return nc


def _plan_and_pack(x, seg):
    """Host-side: chunk boundaries, tile count, packed/padded device arrays."""
    x = np.ascontiguousarray(x, dtype=np.float32)
    seg = np.asarray(seg).astype(np.int64)

    starts = np.searchsorted(seg, np.arange(0, G + 1, S)).astype(np.int64)
    n_rows = np.diff(starts)
    T = max(1, int(-(-int(n_rows.max()) // P)))  # ceil

    counts = np.bincount(seg, minlength=G).astype(np.float64)
    recip = (1.0 / np.maximum(counts, 1.0)).astype(np.float32)

    # row index for [chunk, partition, tile]: row = start_c + t*128 + p
    ridx = (
        starts[:-1][:, None, None]
        + np.arange(P, dtype=np.int64)[None, :, None]
        + (np.arange(T, dtype=np.int64) * P)[None, None, :]
    )
    valid = ridx < starts[1:][:, None, None]
    ridx_c = np.where(valid, ridx, 0)

    xg = x[ridx_c.reshape(-1)].reshape(C_TOTAL, P, T, D)
    xg[~valid] = 0.0
    hi = xg.astype(NP_BF16)
    lo = (xg - hi.astype(np.float32)).astype(NP_BF16)
    xbuf = np.empty((C_TOTAL, P, T, 2 * D), NP_BF16)
    xbuf[..., :D] = hi
    xbuf[..., D:] = lo
    del xg, hi, lo
    xbuf = xbuf.reshape(C_TOTAL, P, T * 2 * D)

    rel_all = (seg % S).astype(np.float32)
    relbuf = np.where(valid, rel_all[ridx_c], np.float32(-1.0)).astype(NP_BF16)

    iota_np = np.tile(np.arange(S, dtype=np.float32), (P, T)).astype(NP_BF16)

    in_maps = []
    for core in range(N_CORES):
        c0, c1 = core * C, (core + 1) * C
        rel_core = relbuf[c0:c1].transpose(1, 0, 2).reshape(P, C * T)
        recip_core = recip[core * C * S : (core + 1) * C * S].reshape(C, S).T
        in_maps.append(
            {
                "x": np.ascontiguousarray(xbuf[c0:c1]),
                "rel": np.ascontiguousarray(rel_core),
                "iota": iota_np,
                "recip": np.ascontiguousarray(recip_core),
            }
        )
    return T, in_maps


def _assemble(results):
    """[core]["out"] of shape [S, C*2*D] -> [G, 2*D]."""
    parts = []
    for core in range(N_CORES):
        v = results[core]["out"].reshape(S, C, 2, D)
        mean = v[:, :, 0, :].transpose(1, 0, 2).reshape(C * S, D)
        ssum = v[:, :, 1, :].transpose(1, 0, 2).reshape(C * S, D)
        parts.append(np.concatenate([mean, ssum], axis=1))
    return np.concatenate(parts, axis=0)


def _run_impl(nbr_fea, segment_ids, num_segments, trace=False, trace_kwargs=None):
    assert int(num_segments) == G, f"expected {G} segments, got {num_segments}"
    assert nbr_fea.shape == (N_TOTAL, D), nbr_fea.shape

    T, in_maps = _plan_and_pack(nbr_fea, segment_ids)
    nc = _build_bass(T)
    kw = {}
    if trace:
        kw = dict(trace=True, **(trace_kwargs or {}))
    res = bass_utils.run_bass_kernel_spmd(
        nc, in_maps, core_ids=list(range(N_CORES)), **kw
    )
    return _assemble(res.results), res


def kernel(nbr_fea, segment_ids, num_segments):
    out, _ = _run_impl(np.asarray(nbr_fea), np.asarray(segment_ids), num_segments)
    return out


# revision 12
# speedup vs baseline: 1.0840x; 1.0226x over previous
"""Trainium2 Bass kernel for sorted-segment sum+mean (segment_reduce).

out[g] = concat(mean_g, sum_g) over rows of nbr_fea grouped by sorted
segment_ids; out shape [num_segments, 2*D].

Strategy
--------
Rows are sorted by segment id, so each segment is a contiguous row range.
Segments are packed greedily into "chunks" of at most S=40 consecutive
segments and at most T*128 rows (T chosen to minimize total padded rows, so
chunks fill to ~98% of capacity); each chunk's rows are packed (on host) into
T row-tiles of 128 rows, laid out DMA-optimally as
[chunk][partition][tile][feat] so chunk loads are fully contiguous DMAs
(two chunks batched per dma_start).

The f32 features are split on host into an exact bf16 hi/lo pair
(x = hi + lo + O(2^-18 x)), shipped side by side — same byte volume as f32 —
so the TensorEngine runs single-pass bf16 matmuls instead of 4x-slower fp32.

On device, per 128-row tile, a one-hot matrix U[row, slot] = (rel_id == slot)
is built on the VectorEngine (is_equal against an iota constant) in bf16 and
used as the matmul *stationary* operand (LDWEIGHTS of S cols, FWL-fast);
the moving operand is the [128 rows, hi|lo = 128] tile:
    psum[slot, 0:64]  += U.T @ hi
    psum[slot, 64:128]+= U.T @ lo
accumulated over the chunk's tiles in PSUM (fp32).  The epilogue adds the two
halves (exact sum; ACT stages the lo half since walrus allows one PSUM operand
per op), scales by host-baked 1/count on ACT for the mean, and stages results
in SBUF, DMA'd out in quarters.  Padding rows carry rel_id = -1 so their
one-hot row is all zero; unused slots of a chunk produce zeros that the host
discards.

The kernel is compiled AFTER seeing the inputs, so the (data-dependent) chunk
plan is a compile-time constant; one SPMD program runs on all 8 cores.
"""

import ml_dtypes
import numpy as np

import concourse.bass as bass
import concourse.mybir as mybir
import concourse.tile as tile
from concourse import bass_utils

N_TOTAL = 4_194_304
D = 64                       # feature dim
G = 32_768                   # num segments
N_CORES = 8
S = 40                       # segment slots per chunk (psum partitions)
P = 128                      # rows per tile == SBUF partitions
DMA_BATCH = 2                # chunks per x dma_start

F32 = mybir.dt.float32
BF16 = mybir.dt.bfloat16
NP_BF16 = ml_dtypes.bfloat16


def _split_syncs(nc, max_waits=1):
    """This container's walrus accepts at most one sync-wait per instruction;
    split extra waits onto preceding same-engine NoOps (engine stalls at each
    wait in turn, so semantics are identical)."""
    n_split = 0
    for f in nc.m.functions:
        for bb in f.blocks:
            new_insts = []
            for ins in bb.instructions:
                si = getattr(ins, "sync_info", None)
                waits = list(si.on_wait) if si is not None and si.on_wait else []
                if len(waits) > max_waits:
                    n_split += 1
                    extra = waits[:-max_waits]
                    for i in range(0, len(extra), max_waits):
                        nop = mybir.InstNoOp(
                            name=f"{ins.name}_wsplit{i}", ins=[], outs=[]
                        )
                        nop.engine = ins.engine
                        nop.sync_info = mybir.SyncInfo(
                            on_wait=extra[i : i + max_waits], on_update=[]
                        )
                        new_insts.append(nop)
                    si.on_wait = waits[-max_waits:]
                new_insts.append(ins)
            bb.instructions = new_insts
    return n_split


def _build_bass(T, C, split_syncs=True):
    """Build the SPMD program: C chunks per core, T row-tiles per chunk."""
    nc = bass.Bass("TRN2", debug=False, num_devices=1)

    x_d = nc.dram_tensor("x", [C, P, T * 2 * D], BF16, kind="ExternalInput")
    rel_d = nc.dram_tensor("rel", [P, C * T], BF16, kind="ExternalInput")
    iota_d = nc.dram_tensor("iota", [P, T * S], BF16, kind="ExternalInput")
    recip_d = nc.dram_tensor("recip", [S, C], F32, kind="ExternalInput")
    out_d = nc.dram_tensor("out", [S, C * 2 * D], F32, kind="ExternalOutput")

    flush_every = -(-C // 4)  # ceil: stage output DMA in ~quarters

    with tile.TileContext(nc) as tc:
        with (
            tc.tile_pool(name="const", bufs=1) as const_pool,
            tc.tile_pool(name="xin", bufs=4) as x_pool,
            tc.tile_pool(name="oh", bufs=3) as oh_pool,
            tc.tile_pool(name="outs", bufs=1) as out_pool,
            tc.tile_pool(name="scr", bufs=3) as scr_pool,
            tc.tile_pool(name="ps", bufs=4, space="PSUM") as ps_pool,
        ):
            # constants go through the SWDGE (gpsimd) queue so the HWDGE
            # rings start streaming x immediately
            rel_sb = const_pool.tile([P, C * T], BF16)
            nc.gpsimd.dma_start(rel_sb[:], rel_d[:])
            iota_sb = const_pool.tile([P, T * S], BF16)
            nc.gpsimd.dma_start(iota_sb[:], iota_d[:])
            recip_sb = const_pool.tile([S, C], F32)
            nc.gpsimd.dma_start(recip_sb[:], recip_d[:])
            out_sb = out_pool.tile([S, C * 2 * D], F32)

            flushed = 0
            xt = None
            for c in range(C):
                b = c % DMA_BATCH
                if b == 0:
                    nb = min(DMA_BATCH, C - c)
                    xt = x_pool.tile([P, nb, T * 2 * D], BF16)
                    dma_eng = nc.sync if (c // DMA_BATCH) % 2 == 0 else nc.scalar
                    dma_eng.dma_start(
                        xt[:],
                        x_d[c : c + nb, :, :].rearrange("c p f -> p c f"),
                    )
                oh = oh_pool.tile([P, T * S], BF16)
                nc.vector.tensor_tensor(
                    oh[:],
                    rel_sb[:, c * T : (c + 1) * T].to_broadcast((P, T, S)),
                    iota_sb[:],
                    mybir.AluOpType.is_equal,
                )
                ps = ps_pool.tile([S, 2 * D], F32)
                for t in range(T):
                    nc.tensor.matmul(
                        ps[:],
                        oh[:, t * S : (t + 1) * S],
                        xt[:, b, t * 2 * D : (t + 1) * 2 * D],
                        start=(t == 0),
                        stop=(t == T - 1),
                    )
                base = c * 2 * D
                # exact sum = hi-part + lo-part (walrus allows only one PSUM
                # operand per op: stage the lo half through SBUF via ACT)
                lo_sb = scr_pool.tile([S, D], F32)
                nc.scalar.copy(lo_sb[:], ps[:, D : 2 * D])
                nc.vector.tensor_tensor(
                    out_sb[:, base + D : base + 2 * D],
                    ps[:, 0:D],
                    lo_sb[:],
                    mybir.AluOpType.add,
                )
                # mean = sum * (1/count)   (per-partition scale, on ACT)
                nc.scalar.activation(
                    out_sb[:, base : base + D],
                    out_sb[:, base + D : base + 2 * D],
                    mybir.ActivationFunctionType.Copy,
                    scale=recip_sb[:, c : c + 1],
                )
                if c + 1 == C or (c + 1) % flush_every == 0:
                    q0 = flushed * 2 * D
                    q1 = (c + 1) * 2 * D
                    nc.sync.dma_start(out_d[:, q0:q1], out_sb[:, q0:q1])
                    flushed = c + 1

    if split_syncs:
        _split_syncs(nc)
    return nc


def _greedy_plan(counts):
    """Pack consecutive segments into chunks with <=S segments and <=T*128
    rows, scanning candidate capacities T to minimize total padded rows.
    Returns (T, bases, nsegs) arrays (unpadded chunk list)."""
    g_total = len(counts)
    t_min = max(1, int(-(-int(counts.max()) // P)))
    # aim for ~32 segments per chunk so the S-slot cap rarely binds
    t_avg = max(t_min, -(-int(counts.sum()) * 32 // (g_total * P)))
    best = None
    for T in range(max(t_min, t_avg - 3), max(t_min, t_avg) + 4):
        cap = T * P
        bases, nsegs = [], []
        g = 0
        r = 0
        n = 0
        while g + n < g_total:
            cnt = counts[g + n]
            if n < S and r + cnt <= cap:
                r += cnt
                n += 1
            else:
                assert n > 0, "single segment exceeds chunk capacity"
                bases.append(g)
                nsegs.append(n)
                g += n
                r = 0
                n = 0
        if n > 0:
            bases.append(g)
            nsegs.append(n)
        ct = len(bases)
        c_per = -(-ct // N_CORES)
        total = c_per * N_CORES * cap
        if best is None or total < best[0]:
            best = (total, T, np.array(bases), np.array(nsegs))
    _, T, bases, nsegs = best
    return T, bases, nsegs


def _plan_and_pack(x, seg):
    """Host-side: greedy chunk plan + packed/padded device arrays."""
    x = np.ascontiguousarray(x, dtype=np.float32)
    seg = np.asarray(seg).astype(np.int64)

    counts = np.bincount(seg, minlength=G).astype(np.int64)
    seg_row_start = np.zeros(G + 1, dtype=np.int64)
    np.cumsum(counts, out=seg_row_start[1:])
    recip = (1.0 / np.maximum(counts, 1.0)).astype(np.float32)

    T, bases, nsegs = _greedy_plan(counts)
    C = -(-len(bases) // N_CORES)  # chunks per core
    ct_pad = C * N_CORES
    pad = ct_pad - len(bases)
    # empty padding chunks (0 segments, 0 rows)
    bases_p = np.concatenate([bases, np.zeros(pad, dtype=np.int64)])
    nsegs_p = np.concatenate([nsegs, np.zeros(pad, dtype=np.int64)])
    row_start = seg_row_start[bases_p]
    n_rows = seg_row_start[bases_p + nsegs_p] - row_start

    # row index for [chunk, partition, tile]: row = start_c + t*128 + p
    ridx = (
        row_start[:, None, None]
        + np.arange(P, dtype=np.int64)[None, :, None]
        + (np.arange(T, dtype=np.int64) * P)[None, None, :]
    )
    valid = ridx < (row_start + n_rows)[:, None, None]
    ridx_c = np.where(valid, ridx, 0)

    xg = x[ridx_c.reshape(-1)].reshape(ct_pad, P, T, D)
    xg[~valid] = 0.0
    hi = xg.astype(NP_BF16)
    lo = (xg - hi.astype(np.float32)).astype(NP_BF16)
    xbuf = np.empty((ct_pad, P, T, 2 * D), NP_BF16)
    xbuf[..., :D] = hi
    xbuf[..., D:] = lo
    del xg, hi, lo
    xbuf = xbuf.reshape(ct_pad, P, T * 2 * D)

    rel = seg[ridx_c] - bases_p[:, None, None]
    relbuf = np.where(valid, rel, -1).astype(NP_BF16)

    iota_np = np.tile(np.arange(S, dtype=np.float32), (P, T)).astype(NP_BF16)

    # per-slot reciprocal: slot s of chunk c -> segment bases[c]+s (1.0 pad)
    gidx = bases_p[:, None] + np.arange(S, dtype=np.int64)[None, :]
    slot_valid = np.arange(S)[None, :] < nsegs_p[:, None]
    recip_slots = np.where(
        slot_valid, recip[np.clip(gidx, 0, G - 1)], np.float32(1.0)
    ).astype(np.float32)

    in_maps = []
    for core in range(N_CORES):
        c0, c1 = core * C, (core + 1) * C
        rel_core = relbuf[c0:c1].transpose(1, 0, 2).reshape(P, C * T)
        in_maps.append(
            {
                "x": np.ascontiguousarray(xbuf[c0:c1]),
                "rel": np.ascontiguousarray(rel_core),
                "iota": iota_np,
                "recip": np.ascontiguousarray(recip_slots[c0:c1].T),
            }
        )
    plan = dict(T=T, C=C, gidx=gidx, slot_valid=slot_valid)
    return plan, in_maps


def _assemble(results, plan):
    """[core]["out"] of shape [S, C*2*D] -> [G, 2*D] via the slot->segment map."""
    C = plan["C"]
    vs = [results[core]["out"].reshape(S, C, 2, D) for core in range(N_CORES)]
    mean = np.concatenate([v[:, :, 0, :].transpose(1, 0, 2) for v in vs])  # [ct,S,D]
    ssum = np.concatenate([v[:, :, 1, :].transpose(1, 0, 2) for v in vs])
    out = np.empty((G, 2 * D), np.float32)
    m = plan["slot_valid"]
    out[plan["gidx"][m], :D] = mean[m]
    out[plan["gidx"][m], D:] = ssum[m]
    return out


def _run_impl(nbr_fea, segment_ids, num_segments, trace=False, trace_kwargs=None):
    assert int(num_segments) == G, f"expected {G} segments, got {num_segments}"
    assert nbr_fea.shape == (N_TOTAL, D), nbr_fea.shape

    plan, in_maps = _plan_and_pack(nbr_fea, segment_ids)
    nc = _build_bass(plan["T"], plan["C"])
    kw = {}
    if trace:
        kw = dict(trace=True, **(trace_kwargs or {}))
    res = bass_utils.run_bass_kernel_spmd(
        nc, in_maps, core_ids=list(range(N_CORES)), **kw
    )
    return _assemble(res.results, plan), res


def kernel(nbr_fea, segment_ids, num_segments):
    out, _ = _run_impl(np.asarray(nbr_fea), np.asarray(segment_ids), num_segments)
    return out
